# revision 1
# baseline (speedup 1.0000x reference)
"""Trainium2 Bass kernel for GQA attention (nn_Attention_15015205667492).

Reference computation (per batch b, seq s=2048, d=2048):
  q = (x @ wq)  -> 32 heads x 64     (RoPE)
  k = (x @ wk)  ->  8 kv heads x 64  (RoPE)
  v = (x @ wv)  ->  8 kv heads x 64
  causal softmax(q k^T / 8) @ v  (GQA: kv head = q head // 4)
  out = attn @ wo

Sharding (8 cores): DP2 x TP4.
  core c: batch = c//4, head-group g = c%4 (Q heads 8g..8g+7, KV heads 2g, 2g+1).
  Each core computes attention for its 8 heads over its batch, writes the
  head-transposed attention output [512, S] bf16 to DRAM, AllGathers it within
  its 4-core batch group -> [2048, S], then computes a column slice of o_proj
  (wo[:, 512g:512(g+1)]) so per-core outputs are disjoint blocks of the final
  output (host-side unshard is pure concatenation).

Kernel layout choices:
  - x is passed transposed+bf16 (xT [d, s]) so all projections contract d on
    partitions.  Q/K come out transposed ([head-pair 128, s]) which is what
    the QK^T matmul wants as lhsT/rhs; V comes out natural ([s, kv 128]).
  - Scores are computed transposed: S^T[k, q] = kT.T @ qT per 128-k-block, so
    softmax numerator exp() runs on ScalarE and the AV matmul consumes P^T
    directly (no P transpose anywhere).
  - Softmax denominator comes free from the AV matmul: V is augmented with a
    ones column, so row 64 of the AV psum accumulates sum_k exp(s); the
    divide uses reciprocal_approx_fast (51 ULP, ample for softmax sums).
  - Causality is static: key blocks beyond the query block are skipped;
    diagonal blocks get a binary mask multiply on P^T, and far-diagonal
    blocks (j>=2) shrink the processed q-window to their visible range.
  - PSUM->SBUF drains run on ScalarE (nc.scalar.copy) to keep VectorE free;
    RoPE's rotate-half is a PE permutation matmul (host-built +-1 matrix)
    because partition-shifted DVE ops are rejected by the compiler.
"""

import sys

sys.path.insert(0, "/opt/trn_rl_repo")

import numpy as np
import ml_dtypes

N_CORES = 8
H, KVH, HD = 32, 8, 64
RG = [[0, 1, 2, 3], [4, 5, 6, 7]]

_cache = {}


def build_program(S=2048, D=2048, enable_asserts=False, NO_CC=False, bench_iters=0, phases=15, ablate=()):
    import concourse.mybir as mybir
    import concourse.tile as tile
    from concourse import bacc

    f32 = mybir.dt.float32
    bf16 = mybir.dt.bfloat16
    Exp = mybir.ActivationFunctionType.Exp

    DC = D // 128       # contraction chunks for projections
    QB = S // 512       # query blocks (512 q rows each)
    KB = S // 128       # key blocks
    DOUT = D // 4       # output column slice per core

    nc = bacc.Bacc(
        "TRN2",
        target_bir_lowering=False,
        debug=False,
        enable_asserts=enable_asserts,
        num_devices=N_CORES,
    )

    xT_d = nc.dram_tensor("xT", [D, S], bf16, kind="ExternalInput")
    wq_d = nc.dram_tensor("wq", [D, 512], bf16, kind="ExternalInput")
    wk_d = nc.dram_tensor("wk", [D, 128], bf16, kind="ExternalInput")
    wv_d = nc.dram_tensor("wv", [D, 128], bf16, kind="ExternalInput")
    wo_d = nc.dram_tensor("wo", [H * HD, DOUT], bf16, kind="ExternalInput")
    cos_d = nc.dram_tensor("cos2", [128, S], bf16, kind="ExternalInput")
    sin_d = nc.dram_tensor("sinsw2", [128, S], bf16, kind="ExternalInput")
    rot_d = nc.dram_tensor("rot", [128, 128], bf16, kind="ExternalInput")
    msk_d = nc.dram_tensor("masks", [128, 4, 1024], bf16, kind="ExternalInput")
    out_d = nc.dram_tensor("out", [S, DOUT], f32, kind="ExternalOutput")

    HC = (H * HD) // 128  # o_proj contraction chunks (16)

    with tile.TileContext(nc) as tc:
        with (
            tc.tile_pool(name="const", bufs=1) as const,
            tc.tile_pool(name="psA", bufs=2, space="PSUM") as psA,
            tc.tile_pool(name="psAV", bufs=1, space="PSUM") as psAV,
            tc.tile_pool(name="psP", bufs=2, space="PSUM") as psP,
            tc.tile_pool(name="work", bufs=2) as work,
            tc.tile_pool(name="dram", bufs=1, space="DRAM") as dram,
        ):
            # ---------------- constants / weights ----------------
            xt = []
            for i in range(DC):
                t = const.tile([128, S], bf16, name=f"xt{i}", tag=f"xt{i}")
                nc.sync.dma_start(out=t[:], in_=xT_d[128 * i : 128 * (i + 1), :])
                xt.append(t)
            wq_t = []
            for i in range(DC):
                t = const.tile([128, 512], bf16, name=f"wq{i}", tag=f"wq{i}")
                nc.sync.dma_start(out=t[:], in_=wq_d[128 * i : 128 * (i + 1), :])
                wq_t.append(t)
            wk_t = []
            wv_t = []
            for i in range(DC):
                t = const.tile([128, 128], bf16, name=f"wk{i}", tag=f"wk{i}")
                nc.sync.dma_start(out=t[:], in_=wk_d[128 * i : 128 * (i + 1), :])
                wk_t.append(t)
                t = const.tile([128, 128], bf16, name=f"wv{i}", tag=f"wv{i}")
                nc.sync.dma_start(out=t[:], in_=wv_d[128 * i : 128 * (i + 1), :])
                wv_t.append(t)
            wo_t = []
            for i in range(HC):
                t = const.tile([128, DOUT], bf16, name=f"wo{i}", tag=f"wo{i}")
                nc.sync.dma_start(out=t[:], in_=wo_d[128 * i : 128 * (i + 1), :])
                wo_t.append(t)
            cos_sb = const.tile([128, S], bf16, name="cos", tag="cos")
            nc.sync.dma_start(out=cos_sb[:], in_=cos_d[:, :])
            sin_sb = const.tile([128, S], bf16, name="sin", tag="sin")
            nc.sync.dma_start(out=sin_sb[:], in_=sin_d[:, :])
            rot_sb = const.tile([128, 128], bf16, name="rot", tag="rot")
            nc.sync.dma_start(out=rot_sb[:], in_=rot_d[:, :])
            msk_sb = const.tile([128, 4, 1024], bf16, name="msk", tag="msk")
            nc.sync.dma_start(out=msk_sb[:], in_=msk_d[:, :, :])
            ones_sb = const.tile([65, 64], f32, name="ones", tag="ones")
            nc.vector.memset(ones_sb[:], 1.0)

            def emit_body():
                # ---------------- Q/K projection + RoPE ----------------
                # RoPE in T-layout: rows = hd index (2 heads stacked), cols = seq.
                # rot-half = partition swap (0:32<->32:64, 64:96<->96:128); the
                # sign lives in the host-prepared sinsw2.
                CH = min(1024, S)
                NC2 = S // CH

                def proj_rope(w_tiles, col0, dest, c2_list=None):
                    # process in 1024-col seq chunks to bound scratch SBUF
                    for c2 in c2_list if c2_list is not None else range(NC2):
                        raw = work.tile([128, CH], bf16, name="raw", tag="raw", bufs=2)
                        tmp = work.tile([128, CH], bf16, name="ropetmp", tag="ropetmp", bufs=2)
                        for q2 in range(CH // 512):
                            qc = (CH // 512) * c2 + q2
                            pq = psP.tile([128, 512], f32, name="pq", tag="pp")
                            for dc in range(DC):
                                nc.tensor.matmul(
                                    pq[:],
                                    w_tiles[dc][:, col0 : col0 + 128],
                                    xt[dc][:, 512 * qc : 512 * (qc + 1)],
                                    start=(dc == 0),
                                    stop=(dc == DC - 1),
                                )
                            nc.scalar.copy(
                                out=raw[:, 512 * q2 : 512 * (q2 + 1)], in_=pq[:]
                            )
                        # rotate-half via PE permutation, sign folded into sinsw2
                        for q2 in range(CH // 512):
                            pr = psP.tile([128, 512], f32, name="pr", tag="pp")
                            nc.tensor.matmul(
                                pr[:],
                                rot_sb[:],
                                raw[:, 512 * q2 : 512 * (q2 + 1)],
                                start=True,
                                stop=True,
                            )
                            nc.vector.tensor_mul(
                                tmp[:, 512 * q2 : 512 * (q2 + 1)],
                                pr[:],
                                sin_sb[:, CH * c2 + 512 * q2 : CH * c2 + 512 * (q2 + 1)],
                            )
                        nc.vector.tensor_mul(
                            raw[:], raw[:], cos_sb[:, CH * c2 : CH * (c2 + 1)]
                        )
                        nc.vector.tensor_add(
                            dest[:, CH * c2 : CH * (c2 + 1)], raw[:], tmp[:]
                        )

                qT = []
                for p in range(4 if phases & 2 else 0):
                    t = const.tile([128, S], bf16, name=f"qT{p}", tag=f"qT{p}")
                    qT.append(t)
                if not (phases & 2):
                    return
                krope = work.tile([128, S], bf16, name="krope", tag="krope", bufs=1)
                proj_rope(wk_t, 0, krope)
                # duplicate each kv head across both 64-partition halves so the
                # two QK matmuls of a head pair land on disjoint PE row groups.
                kTd = []
                for h in range(2):
                    t = const.tile([128, S], bf16, name=f"kTd{h}", tag=f"kTd{h}")
                    nc.sync.dma_start(out=t[0:64, :], in_=krope[64 * h : 64 * h + 64, :])
                    nc.sync.dma_start(out=t[64:128, :], in_=krope[64 * h : 64 * h + 64, :])
                    kTd.append(t)

                # ---------------- V projection (natural layout, +ones col) -----
                v_sb = []
                for kb in range(KB if phases & 1 else 0):
                    vt = const.tile([128, 132], bf16, name=f"v{kb}", tag=f"v{kb}")
                    nc.vector.memset(vt[:, 64:65], 1.0)
                    nc.vector.memset(vt[:, 129:130], 1.0)
                    pv = psP.tile([128, 128], f32, name="pv", tag="pp")
                    for dc in range(DC):
                        nc.tensor.matmul(
                            pv[:],
                            xt[dc][:, 128 * kb : 128 * (kb + 1)],
                            wv_t[dc][:],
                            start=(dc == 0),
                            stop=(dc == DC - 1),
                        )
                    nc.vector.tensor_copy(out=vt[:, 0:64], in_=pv[:, 0:64])
                    nc.vector.tensor_copy(out=vt[:, 65:129], in_=pv[:, 64:128])
                    v_sb.append(vt)

                # ---------------- attention + AllGather + o_proj ----------------
                cc_in = [
                    dram.tile([512, 512], bf16, name=f"cin{qb}", tag=f"cin{qb}")
                    for qb in range(QB)
                ]
                cc_out = [
                    dram.tile([2048, 512], bf16, name=f"cout{qb}", tag=f"cout{qb}")
                    for qb in range(QB)
                ]

                def oproj_emit(qb):
                    cct = []
                    for hc in range(HC):
                        t = work.tile(
                            [128, 512], bf16, name=f"cct{hc}", tag=f"cct{hc}", bufs=1
                        )
                        nc.sync.dma_start(
                            out=t[:], in_=cc_out[qb][128 * hc : 128 * (hc + 1), :]
                        )
                        cct.append(t)
                    for rb in range(4):
                        po = psP.tile([128, DOUT], f32, name="po", tag="pp")
                        for hc in range(HC):
                            nc.tensor.matmul(
                                po[:],
                                cct[hc][:, 128 * rb : 128 * (rb + 1)],
                                wo_t[hc][:],
                                start=(hc == 0),
                                stop=(hc == HC - 1),
                            )
                        ot = work.tile([128, DOUT], f32, name="ot", tag="ot", bufs=1)
                        nc.scalar.copy(out=ot[:], in_=po[:])
                        nc.sync.dma_start(
                            out=out_d[
                                512 * qb + 128 * rb : 512 * qb + 128 * (rb + 1), :
                            ],
                            in_=ot[:],
                        )

                def attn_emit(qb):
                    if not (phases & 4):
                        return
                    kmax = 4 * (qb + 1)
                    for hg in range(2):  # kv head (local)
                        for p2 in range(2):  # head pair within kv group
                            pidx = 2 * hg + p2
                            pav = psAV.tile(
                                [65, 1024], f32, name="pav", tag="pav"
                            )
                            for kb in range(kmax):
                                # diagonal blocks only see queries q >= 128j:
                                # shrink the processed q-window to vw columns
                                j = kb - 4 * qb
                                vw = 512 - 128 * j if j >= 2 else 512
                                q0 = 512 * qb + (512 - vw)
                                ps = psA.tile([128, 1024], f32, name="ps", tag="ps")
                                for i in range(2):
                                    r0 = 64 * i
                                    nc.tensor.matmul(
                                        ps[:, 512 * i : 512 * i + vw],
                                        kTd[hg][r0 : r0 + 64, 128 * kb : 128 * (kb + 1)],
                                        qT[pidx][r0 : r0 + 64, q0 : q0 + vw],
                                        start=True,
                                        stop=True,
                                    )
                                pt = work.tile([128, 1024], bf16, name="pt", tag="pt", bufs=4)
                                if vw == 512:
                                    if "exp" in ablate:
                                        nc.vector.tensor_copy(out=pt[:], in_=ps[:])
                                    else:
                                        nc.scalar.activation(
                                            out=pt[:], in_=ps[:], func=Exp, scale=0.125
                                        )
                                    if j >= 0 and "mask" not in ablate:
                                        nc.vector.tensor_mul(
                                            pt[:], pt[:], msk_sb[:, j, :]
                                        )
                                else:
                                    for i in range(2):
                                        sl = slice(512 * i, 512 * i + vw)
                                        if "exp" in ablate:
                                            nc.vector.tensor_copy(
                                                out=pt[:, sl], in_=ps[:, sl]
                                            )
                                        else:
                                            nc.scalar.activation(
                                                out=pt[:, sl],
                                                in_=ps[:, sl],
                                                func=Exp,
                                                scale=0.125,
                                            )
                                        if "mask" not in ablate:
                                            # restricted tri mask == prefix of mask_0
                                            nc.vector.tensor_mul(
                                                pt[:, sl], pt[:, sl], msk_sb[:, 0, 0:vw]
                                            )
                                for i in range(2):
                                    nc.tensor.matmul(
                                        pav[:, 512 * i + 512 - vw : 512 * (i + 1)],
                                        v_sb[kb][:, 65 * hg : 65 * hg + 65],
                                        pt[:, 512 * i : 512 * i + vw],
                                        start=(kb == 0),
                                        stop=(kb == kmax - 1),
                                    )
                            # normalize: out = O^T_unnorm * (1/colsum) broadcast
                            ou = work.tile([65, 1024], f32, name="ou", tag="ou", bufs=2)
                            nc.scalar.copy(out=ou[:], in_=pav[:])
                            if "norm" in ablate:
                                for i in range(2):
                                    at = work.tile([64, 512], bf16, name="at", tag="at")
                                    nc.vector.tensor_copy(
                                        out=at[:], in_=ou[0:64, 512 * i : 512 * (i + 1)]
                                    )
                                    nc.sync.dma_start(
                                        out=cc_in[qb][
                                            128 * pidx + 64 * i : 128 * pidx + 64 * (i + 1), :
                                        ],
                                        in_=at[:],
                                    )
                                continue_norm = False
                            else:
                                continue_norm = True
                            if continue_norm:
                                rbc = work.tile([64, 1024], f32, name="rbc", tag="rbc")
                                for i in range(2):
                                    pb = psP.tile([64, 512], f32, name=f"pb{i}", tag="pp")
                                    nc.tensor.matmul(
                                        pb[:],
                                        ones_sb[64:65, :],
                                        ou[64:65, 512 * i : 512 * (i + 1)],
                                        start=True,
                                        stop=True,
                                    )
                                    nc.vector.reciprocal_approx_fast(
                                        out=rbc[:, 512 * i : 512 * (i + 1)], in_=pb[:]
                                    )
                                at = work.tile([64, 1024], bf16, name="at", tag="at")
                                nc.vector.tensor_mul(at[:], ou[0:64, :], rbc[:])
                                for i in range(2):
                                    nc.sync.dma_start(
                                        out=cc_in[qb][
                                            128 * pidx + 64 * i : 128 * pidx + 64 * (i + 1), :
                                        ],
                                        in_=at[:, 512 * i : 512 * (i + 1)],
                                    )
                    if phases & 8:
                        if NO_CC:
                            nc.sync.dma_start(
                                out=cc_out[qb][0:512, :], in_=cc_in[qb][:, :]
                            )
                        else:
                            nc.gpsimd.collective_compute(
                                "AllGather",
                                mybir.AluOpType.bypass,
                                replica_groups=RG,
                                ins=[cc_in[qb].opt()],
                                outs=[cc_out[qb].opt()],
                            )

                for c2 in range(NC2):
                    for p in range(4 if phases & 2 else 0):
                        proj_rope(wq_t, 128 * p, qT[p], c2_list=[c2])
                    for qb in range(QB):
                        if (512 * qb) // CH == c2:
                            attn_emit(qb)
                if phases & 8 and phases & 4:
                    for qb in range(QB):
                        oproj_emit(qb)

            if bench_iters:
                with tc.For_i(0, bench_iters, 1, name="bench"):
                    emit_body()
            else:
                emit_body()

    nc.compile()
    return nc


def prep_inputs(x, cos, sin, wq, wk, wv, wo):
    """Shard + reformat full inputs into per-core input maps."""
    bf = ml_dtypes.bfloat16
    b, s, d = x.shape
    dout = d // 4
    cos2 = np.tile(np.ascontiguousarray(cos.T), (2, 1)).astype(bf)
    sinT = np.ascontiguousarray(sin.T)
    sinsw = np.concatenate([-sinT[:32], sinT[32:]], axis=0)
    sinsw2 = np.tile(sinsw, (2, 1)).astype(bf)
    # rotate-half permutation: tmp[i] = raw[sigma(i)]; out = R.T @ raw
    rotm = np.zeros((128, 128), np.float32)
    for i in range(128):
        j = (i // 64) * 64 + ((i % 64) + 32) % 64
        rotm[j, i] = 1.0
    rotm = rotm.astype(bf)
    k_loc = np.arange(128)[:, None]
    q_loc = np.arange(512)[None, :]
    ms = []
    for j in range(4):
        mj = (k_loc <= q_loc - 128 * j).astype(np.float32)
        ms.append(np.concatenate([mj, mj], axis=1))
    masks = np.stack(ms, axis=1).astype(bf)  # [128, 4, 1024]

    in_maps = []
    for c in range(N_CORES):
        bb, g = divmod(c, 4)
        in_maps.append(
            {
                "xT": np.ascontiguousarray(x[bb].T).astype(bf),
                "wq": np.ascontiguousarray(wq[:, 512 * g : 512 * (g + 1)]).astype(bf),
                "wk": np.ascontiguousarray(wk[:, 128 * g : 128 * (g + 1)]).astype(bf),
                "wv": np.ascontiguousarray(wv[:, 128 * g : 128 * (g + 1)]).astype(bf),
                "wo": np.ascontiguousarray(wo[:, dout * g : dout * (g + 1)]).astype(bf),
                "cos2": cos2,
                "sinsw2": sinsw2,
                "rot": rotm,
                "masks": masks,
            }
        )
    return in_maps


def assemble_output(results, b, s, d):
    full = np.empty((b, s, d), np.float32)
    dout = d // 4
    for c in range(N_CORES):
        bb, g = divmod(c, 4)
        full[bb][:, dout * g : dout * (g + 1)] = results[c]["out"]
    return full


def kernel(**inputs):
    x = np.asarray(inputs["x"], np.float32)
    b, s, d = x.shape
    key = (s, d)
    if key not in _cache:
        _cache[key] = build_program(S=s, D=d)
    nc = _cache[key]
    in_maps = prep_inputs(
        x,
        np.asarray(inputs["cos"], np.float32),
        np.asarray(inputs["sin"], np.float32),
        np.asarray(inputs["wq"], np.float32),
        np.asarray(inputs["wk"], np.float32),
        np.asarray(inputs["wv"], np.float32),
        np.asarray(inputs["wo"], np.float32),
    )
    from concourse.bass_utils import run_bass_kernel_spmd

    res = run_bass_kernel_spmd(nc, in_maps, core_ids=list(range(N_CORES)))
    return assemble_output(res.results, b, s, d)



# revision 24
# speedup vs baseline: 15.4609x; 15.4609x over previous
"""Trainium2 Bass kernel for GQA attention (nn_Attention_15015205667492).

Reference computation (per batch b, seq s=2048, d=2048):
  q = (x @ wq)  -> 32 heads x 64     (RoPE)
  k = (x @ wk)  ->  8 kv heads x 64  (RoPE)
  v = (x @ wv)  ->  8 kv heads x 64
  causal softmax(q k^T / 8) @ v  (GQA: kv head = q head // 4)
  out = attn @ wo

Sharding (8 cores): DP2 x TP4.
  core c: batch = c//4, head-group g = c%4 (Q heads 8g..8g+7, KV heads 2g, 2g+1).
  Each core computes attention for its 8 heads over its batch, writes the
  head-transposed attention output [512, S] bf16 to DRAM, AllGathers it within
  its 4-core batch group -> [2048, S], then computes a column slice of o_proj
  (wo[:, 512g:512(g+1)]) so per-core outputs are disjoint blocks of the final
  output (host-side unshard is pure concatenation).

v2 schedule: one long interleaved emission stream so the per-engine static
order keeps TensorE dense: V proj, K proj, Q proj(cols 0:1024), then the
attention query-blocks with Q proj(cols 1024:2048) quanta injected into
qb0/qb1 and o_proj quanta (per 128-row slice) injected into qb2/qb3 as each
AllGather lands.  Scores/exp/AV use a [128, 2, 512] layout; the causal mask
multiply is a [128,2,128] op on the first 128 cols of each block's visible
window (blocks left of the diagonal shrink their window by 128*j).  The
softmax denominator rides row 64 of the AV psum (ones column in V); the
colsum broadcast matmuls run in bf16.  PSUM drains run on whichever of
ScalarE/VectorE is off the critical path (exp lives on ScalarE).
"""

import sys

sys.path.insert(0, "/opt/trn_rl_repo")

import numpy as np
import ml_dtypes

N_CORES = 8
H, KVH, HD = 32, 8, 64
RG = [[0, 1, 2, 3], [4, 5, 6, 7]]

_cache = {}


def build_program(S=2048, D=2048, enable_asserts=False, NO_CC=False, bench_iters=0):
    import concourse.mybir as mybir
    import concourse.tile as tile
    from concourse import bacc

    f32 = mybir.dt.float32
    bf16 = mybir.dt.bfloat16
    fp8 = mybir.dt.float8e4
    DR = mybir.MatmulPerfMode.DoubleRow
    Exp = mybir.ActivationFunctionType.Exp

    DC = D // 128       # contraction chunks for projections
    QB = S // 512       # query blocks (512 q rows each)
    KB = S // 128       # key blocks
    DOUT = D // 4       # output column slice per core
    HC = (H * HD) // 128  # o_proj contraction chunks (16)

    nc = bacc.Bacc(
        "TRN2",
        target_bir_lowering=False,
        debug=False,
        enable_asserts=enable_asserts,
        num_devices=N_CORES,
    )

    xT_d = nc.dram_tensor("xT", [DC, 128, S], bf16, kind="ExternalInput")
    wq_d = nc.dram_tensor("wq", [DC, 128, 512], bf16, kind="ExternalInput")
    wk_d = nc.dram_tensor("wk", [DC, 128, 128], bf16, kind="ExternalInput")
    wv_d = nc.dram_tensor("wv", [DC, 128, 128], bf16, kind="ExternalInput")
    wo_d = nc.dram_tensor("wo", [H * HD, DOUT], bf16, kind="ExternalInput")
    cos_d = nc.dram_tensor("cos2", [128, S], bf16, kind="ExternalInput")
    sin_d = nc.dram_tensor("sinsw2", [128, S], bf16, kind="ExternalInput")
    rot_d = nc.dram_tensor("rot", [128, 128], bf16, kind="ExternalInput")
    msk_d = nc.dram_tensor("masks", [128, 128], bf16, kind="ExternalInput")
    out_d = nc.dram_tensor("out", [S, DOUT], f32, kind="ExternalOutput")

    with tile.TileContext(nc) as tc:
        with (
            tc.tile_pool(name="const", bufs=1) as const,
            tc.tile_pool(name="psA", bufs=2, space="PSUM") as psA,
            tc.tile_pool(name="psAV", bufs=1, space="PSUM") as psAV,
            tc.tile_pool(name="psP", bufs=2, space="PSUM") as psP,
            tc.tile_pool(name="work", bufs=2) as work,
            tc.tile_pool(name="dram", bufs=1, space="DRAM") as dram,
        ):
            # ---------------- constants / weights ----------------
            xt8 = const.tile([128, DC, S], bf16, name="xt8", tag="xt8")
            wq8 = const.tile([128, DC, 512], bf16, name="wq8", tag="wq8")
            wk8 = const.tile([128, DC, 128], bf16, name="wk8", tag="wk8")
            wv8 = const.tile([128, DC, 128], bf16, name="wv8", tag="wv8")
            for i in range(DC):
                nc.sync.dma_start(out=xt8[:, i, :], in_=xT_d[i, :, :])
                nc.sync.dma_start(out=wq8[:, i, :], in_=wq_d[i, :, :])
                nc.sync.dma_start(out=wk8[:, i, :], in_=wk_d[i, :, :])
                nc.sync.dma_start(out=wv8[:, i, :], in_=wv_d[i, :, :])
            wo_t = []
            for i in range(HC):
                t = const.tile([128, DOUT], bf16, name=f"wo{i}", tag=f"wo{i}")
                nc.sync.dma_start(out=t[:], in_=wo_d[128 * i : 128 * (i + 1), :])
                wo_t.append(t)
            cos_sb = const.tile([128, S], bf16, name="cos", tag="cos")
            nc.sync.dma_start(out=cos_sb[:], in_=cos_d[:, :])
            sin_sb = const.tile([128, S], bf16, name="sin", tag="sin")
            nc.sync.dma_start(out=sin_sb[:], in_=sin_d[:, :])
            rot_sb = const.tile([128, 128], bf16, name="rot", tag="rot")
            nc.sync.dma_start(out=rot_sb[:], in_=rot_d[:, :])
            msk_sb = const.tile([128, 128], bf16, name="msk", tag="msk")
            nc.sync.dma_start(out=msk_sb[:], in_=msk_d[:, :])
            ones_sb = const.tile([65, 64], bf16, name="ones", tag="ones")
            nc.vector.memset(ones_sb[:], 1.0)

            def emit_body():
                CH = min(1024, S)
                NC2 = S // CH

                qT = [
                    const.tile([128, S], bf16, name=f"qT{p}", tag=f"qT{p}")
                    for p in range(4)
                ]
                kTd = [
                    const.tile([128, S], bf16, name=f"kTd{h}", tag=f"kTd{h}")
                    for h in range(2)
                ]
                v_sb = [
                    const.tile([128, 132], bf16, name=f"v{kb}", tag=f"v{kb}")
                    for kb in range(KB)
                ]
                cc_in = [
                    dram.tile([512, 512], bf16, name=f"cin{qb}", tag=f"cin{qb}")
                    for qb in range(QB)
                ]
                cc_out = [
                    dram.tile([2048, 512], bf16, name=f"cout{qb}", tag=f"cout{qb}")
                    for qb in range(QB)
                ]

                # ---------------- projections + RoPE ----------------
                # RoPE in T-layout: rows = hd index (2 heads stacked), cols =
                # seq; rot-half is a PE permutation (sign folded into sinsw2).
                def proj_rope(w_tiles, col0, dest, c2, dest_split=None):
                    raw = work.tile([128, CH], bf16, name="raw", tag="raw", bufs=2)
                    tmp = work.tile([128, CH], bf16, name="ropetmp", tag="ropetmp", bufs=2)
                    pq = [
                        psP.tile([128, 512], f32, name=f"pq{q2}", tag="pp")
                        for q2 in range(2)
                    ]
                    # one weights load feeds both 512-col windows
                    for dc in range(DC):
                        for q2 in range(2):
                            qc = 2 * c2 + q2
                            nc.tensor.matmul(
                                pq[q2][:],
                                w_tiles[:, dc, col0 : col0 + 128],
                                xt8[:, dc, 512 * qc : 512 * (qc + 1)],
                                start=(dc == 0),
                                stop=(dc == DC - 1),
                                skip_group_check=True,
                            )
                    for q2 in range(2):
                        nc.vector.tensor_copy(
                            out=raw[:, 512 * q2 : 512 * (q2 + 1)], in_=pq[q2][:]
                        )
                    for q2 in range(2):
                        pr = psP.tile([128, 512], f32, name="pr", tag="pp")
                        nc.tensor.matmul(
                            pr[:],
                            rot_sb[:],
                            raw[:, 512 * q2 : 512 * (q2 + 1)],
                            start=True,
                            stop=True,
                        )
                        nc.vector.tensor_mul(
                            tmp[:, 512 * q2 : 512 * (q2 + 1)],
                            pr[:],
                            sin_sb[:, CH * c2 + 512 * q2 : CH * c2 + 512 * (q2 + 1)],
                        )
                    nc.vector.tensor_mul(
                        raw[:], raw[:], cos_sb[:, CH * c2 : CH * (c2 + 1)]
                    )
                    if dest_split is None:
                        nc.vector.tensor_add(
                            dest[:, CH * c2 : CH * (c2 + 1)], raw[:], tmp[:]
                        )
                    else:
                        # K proj: head h's dims (rows 64h:64h+64) land in the
                        # top half of kTd[h]; the bottom half is a dup (DMA'd)
                        for h in range(2):
                            nc.vector.tensor_add(
                                dest_split[h][0:64, CH * c2 : CH * (c2 + 1)],
                                raw[64 * h : 64 * h + 64, :],
                                tmp[64 * h : 64 * h + 64, :],
                            )

                def vproj(kb):
                    vt = v_sb[kb]
                    nc.vector.memset(vt[:, 64:65], 1.0)
                    nc.vector.memset(vt[:, 129:130], 1.0)
                    pv = psP.tile([128, 128], f32, name="pv", tag="pp")
                    for dc in range(DC):
                        nc.tensor.matmul(
                            pv[:],
                            xt8[:, dc, 128 * kb : 128 * (kb + 1)],
                            wv8[:, dc, :],
                            start=(dc == 0),
                            stop=(dc == DC - 1),
                        )
                    nc.vector.tensor_copy(out=vt[:, 0:64], in_=pv[:, 0:64])
                    nc.vector.tensor_copy(out=vt[:, 65:129], in_=pv[:, 64:128])

                # ---------------- attention ----------------
                def attn_pair(qb, pidx):
                    hg = pidx // 2
                    kmax = 4 * (qb + 1)
                    pav = psAV.tile([65, 1024], f32, name="pav", tag="pav")

                    def emit_av(kb, pt, vw):
                        for i in range(2):
                            nc.tensor.matmul(
                                pav[:, 512 * i + 512 - vw : 512 * (i + 1)],
                                v_sb[kb][:, 65 * hg : 65 * hg + 65],
                                pt[:, i, 0:vw],
                                start=(kb == 0),
                                stop=(kb == kmax - 1),
                            )

                    pend = None  # software-pipeline: AV lags scores/exp by one
                    for kb in range(kmax):
                        j = kb - 4 * qb
                        vw = 512 - 128 * j if j >= 0 else 512
                        q0 = 512 * qb + (512 - vw)
                        ps = psA.tile([128, 2, 512], f32, name="ps", tag="ps")
                        for i in range(2):
                            r0 = 64 * i
                            nc.tensor.matmul(
                                ps[:, i, 0:vw],
                                kTd[hg][r0 : r0 + 64, 128 * kb : 128 * (kb + 1)],
                                qT[pidx][r0 : r0 + 64, q0 : q0 + vw],
                                start=True,
                                stop=True,
                            )
                        pt = work.tile([128, 2, 512], bf16, name="pt", tag="pt", bufs=4)
                        nc.scalar.activation(
                            out=pt[:, :, 0:vw], in_=ps[:, :, 0:vw], func=Exp, scale=0.125
                        )
                        if j >= 0:
                            # only the first 128 cols of the visible window are
                            # on the diagonal; the rest is fully visible
                            for i in range(2):
                                nc.vector.tensor_mul(
                                    pt[:, i, 0:128], pt[:, i, 0:128], msk_sb[:, 0:128]
                                )
                        emit_av(kb, pt, vw)
                    # normalize: out = num * (1/colsum), colsum broadcast via PE
                    ou = work.tile([65, 1024], bf16, name="ou", tag="ou", bufs=2)
                    nc.vector.tensor_copy(out=ou[:], in_=pav[:])
                    at = work.tile([64, 1024], bf16, name="at", tag="at")
                    for i in range(2):
                        pb = psP.tile([64, 512], f32, name=f"pb{i}", tag="pp")
                        nc.tensor.matmul(
                            pb[:],
                            ones_sb[64:65, :],
                            ou[64:65, 512 * i : 512 * (i + 1)],
                            start=True,
                            stop=True,
                        )
                        rbc = work.tile([64, 512], f32, name="rbc", tag="rbc", bufs=2)
                        nc.vector.reciprocal_approx_fast(out=rbc[:], in_=pb[:])
                        nc.vector.tensor_mul(
                            at[:, 512 * i : 512 * (i + 1)],
                            ou[0:64, 512 * i : 512 * (i + 1)],
                            rbc[:],
                        )
                    for i in range(2):
                        nc.sync.dma_start(
                            out=cc_in[qb][
                                128 * pidx + 64 * i : 128 * pidx + 64 * (i + 1), :
                            ],
                            in_=at[:, 512 * i : 512 * (i + 1)],
                        )

                def allgather(qb):
                    if NO_CC:
                        nc.sync.dma_start(out=cc_out[qb][0:512, :], in_=cc_in[qb][:, :])
                    else:
                        nc.gpsimd.collective_compute(
                            "AllGather",
                            mybir.AluOpType.bypass,
                            replica_groups=RG,
                            ins=[cc_in[qb].opt()],
                            outs=[cc_out[qb].opt()],
                        )

                # ---------------- o_proj quanta ----------------
                def oproj_quanta(qb):
                    cct = []

                    def load(lo, hi):
                        def _f():
                            for hc in range(lo, hi):
                                t = work.tile(
                                    [128, 512], bf16, name=f"cct{hc}", tag=f"cct{hc}",
                                    bufs=2,
                                )
                                nc.sync.dma_start(
                                    out=t[:],
                                    in_=cc_out[qb][128 * hc : 128 * (hc + 1), :],
                                )
                                cct.append(t)
                        return _f

                    def rb_quant(rb):
                        def _f():
                            po = psP.tile([128, DOUT], f32, name="po", tag="pp")
                            for hc in range(HC):
                                nc.tensor.matmul(
                                    po[:],
                                    cct[hc][:, 128 * rb : 128 * (rb + 1)],
                                    wo_t[hc][:],
                                    start=(hc == 0),
                                    stop=(hc == HC - 1),
                                )
                            ot = work.tile([128, DOUT], f32, name="ot", tag="ot", bufs=2)
                            nc.vector.tensor_copy(out=ot[:], in_=po[:])
                            nc.sync.dma_start(
                                out=out_d[
                                    512 * qb + 128 * rb : 512 * qb + 128 * (rb + 1), :
                                ],
                                in_=ot[:],
                            )
                        return _f

                    return [load(0, 8), load(8, 16)] + [rb_quant(rb) for rb in range(4)]

                # ---------------- the interleaved schedule ----------------
                for kb in range(KB):
                    vproj(kb)
                proj_rope(wk8, 0, None, 0, dest_split=kTd)
                proj_rope(wk8, 0, None, 1, dest_split=kTd)
                for h in range(2):
                    nc.sync.dma_start(out=kTd[h][64:128, :], in_=kTd[h][0:64, :])
                for p in range(4):
                    proj_rope(wq8, 128 * p, qT[p], 0)

                oq = {}
                # injections: (qb, pidx) -> list of emission closures
                inj = {
                    (0, 1): [lambda: proj_rope(wq8, 0, qT[0], 1)],
                    (0, 3): [lambda: proj_rope(wq8, 128, qT[1], 1)],
                    (1, 1): [lambda: proj_rope(wq8, 256, qT[2], 1)],
                    (1, 3): [lambda: proj_rope(wq8, 384, qT[3], 1)],
                }

                for qb in range(QB):
                    if qb == 2:
                        oq[0] = oproj_quanta(0)
                        inj[(2, 0)] = [oq[0][0]]
                        inj[(2, 1)] = [oq[0][1], oq[0][2]]
                        inj[(2, 2)] = [oq[0][3], oq[0][4]]
                        inj[(2, 3)] = [oq[0][5]]
                    if qb == 3:
                        oq[1] = oproj_quanta(1)
                        oq[2] = oproj_quanta(2)
                        inj[(3, 0)] = oq[1][0:3]
                        inj[(3, 1)] = oq[1][3:6]
                        inj[(3, 2)] = oq[2][0:3]
                        inj[(3, 3)] = oq[2][3:6]
                    for pidx in range(4):
                        attn_pair(qb, pidx)
                        for f in inj.get((qb, pidx), []):
                            f()
                    allgather(qb)
                for f in oproj_quanta(3):
                    f()

            if bench_iters:
                with tc.For_i(0, bench_iters, 1, name="bench"):
                    emit_body()
            else:
                emit_body()

    nc.compile()
    return nc


def _to_fp8(a, scale):
    """Scale, clip to TRN e4m3 range (+-240), cast."""
    return np.clip(a * scale, -240.0, 240.0).astype(ml_dtypes.float8_e4m3)


def prep_inputs(x, cos, sin, wq, wk, wv, wo):
    """Shard + reformat full inputs into per-core input maps.

    fp8 scaling: x8 = 8x, w{q,k,v}8 = 64w -> projections come out x512; the
    512 is folded into cos2/sinsw2 for Q/K (RoPE output unscaled) and into
    the V psum drain for V.
    """
    bf = ml_dtypes.bfloat16
    b, s, d = x.shape
    dout = d // 4
    cos2 = np.tile(np.ascontiguousarray(cos.T), (2, 1)).astype(bf)
    sinT = np.ascontiguousarray(sin.T)
    sinsw = np.concatenate([-sinT[:32], sinT[32:]], axis=0)
    sinsw2 = np.tile(sinsw, (2, 1)).astype(bf)
    # rotate-half permutation: tmp[i] = raw[sigma(i)]; out = R.T @ raw
    rotm = np.zeros((128, 128), np.float32)
    for i in range(128):
        j = (i // 64) * 64 + ((i % 64) + 32) % 64
        rotm[j, i] = 1.0
    rotm = rotm.astype(bf)
    k_loc = np.arange(128)[:, None]
    q_loc = np.arange(128)[None, :]
    masks = (k_loc <= q_loc).astype(np.float32).astype(bf)  # [128,128]

    dc = d // 128
    in_maps = []
    for c in range(N_CORES):
        bb, g = divmod(c, 4)
        in_maps.append(
            {
                "xT": np.ascontiguousarray(x[bb].T.reshape(dc, 128, s)).astype(bf),
                "wq": np.ascontiguousarray(
                    wq[:, 512 * g : 512 * (g + 1)].reshape(dc, 128, 512)
                ).astype(bf),
                "wk": np.ascontiguousarray(
                    wk[:, 128 * g : 128 * (g + 1)].reshape(dc, 128, 128)
                ).astype(bf),
                "wv": np.ascontiguousarray(
                    wv[:, 128 * g : 128 * (g + 1)].reshape(dc, 128, 128)
                ).astype(bf),
                "wo": np.ascontiguousarray(wo[:, dout * g : dout * (g + 1)]).astype(bf),
                "cos2": cos2,
                "sinsw2": sinsw2,
                "rot": rotm,
                "masks": masks,
            }
        )
    return in_maps


def assemble_output(results, b, s, d):
    full = np.empty((b, s, d), np.float32)
    dout = d // 4
    for c in range(N_CORES):
        bb, g = divmod(c, 4)
        full[bb][:, dout * g : dout * (g + 1)] = results[c]["out"]
    return full


def kernel(**inputs):
    x = np.asarray(inputs["x"], np.float32)
    b, s, d = x.shape
    key = (s, d)
    if key not in _cache:
        _cache[key] = build_program(S=s, D=d)
    nc = _cache[key]
    in_maps = prep_inputs(
        x,
        np.asarray(inputs["cos"], np.float32),
        np.asarray(inputs["sin"], np.float32),
        np.asarray(inputs["wq"], np.float32),
        np.asarray(inputs["wk"], np.float32),
        np.asarray(inputs["wv"], np.float32),
        np.asarray(inputs["wo"], np.float32),
    )
    from concourse.bass_utils import run_bass_kernel_spmd

    res = run_bass_kernel_spmd(nc, in_maps, core_ids=list(range(N_CORES)))
    return assemble_output(res.results, b, s, d)
